# revision 38
# baseline (speedup 1.0000x reference)
"""Trainium2 Bass kernel for nn_Acts2LayoutModel (gnn_message_passing).

Strategy: pure data-parallel over batch (16 batch / 8 cores = 2 per core,
no collectives). All graph gathers/scatters are expressed as one-hot
matmuls with host-precomputed index matrices; all dense layers run in a
feature-major ("transposed") layout so no on-device transposes are needed.
Matmul operands are bf16 (f32 PSUM accumulation); the box recurrence is f32.
"""

import numpy as np
import ml_dtypes

BF = ml_dtypes.bfloat16

# problem shapes (hardcoded per spec)
B, F, O, T, A = 16, 12, 96, 160, 64
D, H = 128, 512
NL = 5
NCORES = 8
BL = B // NCORES          # 2 batch elems per core
OC = BL * O               # 192 object columns
EPB = T + A               # 224 edges per batch elem
EC = BL * EPB             # 448 edge columns
NT = F - 1                # 11 recurrent steps
OBJ_IN = 3 * D            # 384

_NC_CACHE = None
LAST_RESULT = None        # BassKernelResults of the most recent run


# ----------------------------------------------------------------------------
# host-side input preparation
# ----------------------------------------------------------------------------

def _prep_inputs(objs, triplets, actions, boxes_gt, params):
    objs = np.asarray(objs)
    trip = np.asarray(triplets)
    actions = np.asarray(actions, np.float32)
    boxes_gt = np.asarray(boxes_gt, np.float32)

    attr = [np.asarray(params["attr_emb"][i], np.float32) for i in range(3)]
    pred_emb = np.asarray(params["pred_emb"], np.float32)
    acts_emb = np.asarray(params["acts_emb"], np.float32)
    gconv = [{k: np.asarray(v, np.float32) for k, v in lp.items()}
             for lp in params["gconv"]]
    objW1 = np.asarray(params["obj_W1"], np.float32)
    objW2 = np.asarray(params["obj_W2"], np.float32)
    boxW1 = np.asarray(params["box_W1"], np.float32)
    boxW2 = np.asarray(params["box_W2"], np.float32)
    boxb1 = np.asarray(params["box_b1"], np.float32)
    boxb2 = np.asarray(params["box_b2"], np.float32)

    sa = actions[:, :, 0].astype(np.int64)
    a_f = actions[:, :, 1]
    oa = actions[:, :, 2].astype(np.int64)
    f1, f2 = actions[:, :, 3], actions[:, :, 4]
    xe, ye = actions[:, :, 5], actions[:, :, 6]

    tarr = np.arange(F, dtype=np.float32)
    rt = tarr[None, :, None] / F * (f2 - f1 + 1e-6)[:, None, :] + f1[:, None, :]
    inc = (rt >= 0.0) & (rt <= 1.0)
    a_m = np.where(inc, a_f[:, None, :], 0.0)               # (B,F,A)

    # per-step (t = 1..F-1) edge indices
    s_sp, p_sp, o_sp = trip[:, 1:, :, 0], trip[:, 1:, :, 1], trip[:, 1:, :, 2]
    am_t = a_m[:, 1:].astype(np.int64)                       # (B,NT,A)
    s_all = np.concatenate(
        [s_sp, np.broadcast_to(sa[:, None], (B, NT, A))], 2)  # (B,NT,EPB)
    o_all = np.concatenate(
        [o_sp, np.broadcast_to(oa[:, None], (B, NT, A))], 2)
    ind = np.concatenate([p_sp != 0, am_t != 0], 2).astype(np.float32)

    oh_s = (s_all[..., None] == np.arange(O)).astype(np.float32)  # (B,NT,EPB,O)
    oh_o = (o_all[..., None] == np.arange(O)).astype(np.float32)
    cnt = (oh_s * ind[..., None]).sum(2) + (oh_o * ind[..., None]).sum(2)
    scale = 1.0 / np.maximum(cnt, 1.0)                        # (B,NT,O)
    Hs = oh_s * ind[..., None] * scale[:, :, None, :]
    Ho = oh_o * ind[..., None] * scale[:, :, None, :]

    def to_bf(x):
        return np.ascontiguousarray(x.astype(BF))

    # gather matrices, [o, e] layout: (core, t, O, EC)
    def g_dev(oh):
        g = oh.transpose(0, 1, 3, 2)                          # (B,NT,O,EPB)
        g = g.reshape(NCORES, BL, NT, O, EPB).transpose(0, 2, 3, 1, 4)
        return to_bf(g.reshape(NCORES, NT, O, EC))

    gs_dev, go_dev = g_dev(oh_s), g_dev(oh_o)

    # scatter matrices, rows = edge-chunk of 112, col block g=(b,ec): (core,t,112,4*O)
    def h_dev(hh):
        h = hh.reshape(NCORES, BL, NT, 2, 112, O).transpose(0, 2, 4, 1, 3, 5)
        return to_bf(h.reshape(NCORES, NT, 112, 4 * O))

    hs_dev, ho_dev = h_dev(Hs), h_dev(Ho)

    # layer-0 predicate/action vectors, transposed: (core, t, D, EC)
    pv = pred_emb[p_sp]                                       # (B,NT,T,D)
    av = acts_emb[am_t].copy()                                # (B,NT,A,D)
    av[..., D - 3] = np.broadcast_to(xe[:, None], (B, NT, A))
    av[..., D - 2] = np.broadcast_to(ye[:, None], (B, NT, A))
    av[..., D - 1] = rt[:, 1:]
    pvs = np.concatenate([pv, av], 2)                         # (B,NT,EPB,D)
    pvs = pvs.reshape(NCORES, BL, NT, EPB, D).transpose(0, 2, 4, 1, 3)
    pvs_dev = to_bf(pvs.reshape(NCORES, NT, D, EC))

    # object attribute embeddings, transposed: (core, 3, 128, OC)
    obj_emb = np.concatenate([attr[i][objs[..., i]] for i in range(3)], -1)
    oe = obj_emb.reshape(NCORES, BL, O, OBJ_IN).transpose(0, 3, 1, 2)
    oe_dev = to_bf(oe.reshape(NCORES, OBJ_IN, OC).reshape(NCORES, 3, 128, OC))

    # initial boxes, transposed f32: (core, 4, OC)
    bx0 = boxes_gt[:, 0].reshape(NCORES, BL, O, 4).transpose(0, 3, 1, 2)
    bx0_dev = np.ascontiguousarray(bx0.reshape(NCORES, 4, OC), np.float32)

    # weights (replicated across cores)
    w = {}
    w["objW1"] = to_bf(objW1[:OBJ_IN].reshape(3, 128, OBJ_IN))
    w["objW1b"] = to_bf(objW1[OBJ_IN:])
    w["objW2"] = to_bf(objW2.reshape(3, 128, OBJ_IN))
    for l, lp in enumerate(gconv):
        nk = 7 if l == 0 else 3
        w[f"n1w1_{l}"] = to_bf(lp["n1W1"].reshape(nk, 128, H))
        w[f"n1w2_{l}"] = to_bf(lp["n1W2"].reshape(4, 128, 2 * H + D))
        w[f"n2w1_{l}"] = to_bf(lp["n2W1"].reshape(4, 128, H))
        w[f"n2w2_{l}"] = to_bf(lp["n2W2"].reshape(4, 128, D))
    w["boxw1"] = to_bf(boxW1)
    w["boxw2"] = to_bf(boxW2.reshape(4, 128, 4))

    # partition-wise biases: (128, 55) f32
    bias_p = np.zeros((128, 10 * NL + 5), np.float32)
    for l, lp in enumerate(gconv):
        bias_p[:, l * 10:l * 10 + 4] = lp["n1b1"].reshape(4, 128).T
        bias_p[:, l * 10 + 4] = lp["n1b2"][H:H + D]
        bias_p[:, l * 10 + 5:l * 10 + 9] = lp["n2b1"].reshape(4, 128).T
        bias_p[:, l * 10 + 9] = lp["n2b2"]
    bias_p[:, 50:54] = boxb1.reshape(4, 128).T
    bias_p[:4, 54] = boxb2
    bias_p = np.ascontiguousarray(bias_p)

    # free-dim biases, one row each (added via K=1 ones-row matmuls when any
    # is nonzero; the matmuls are skipped entirely otherwise): (NL,2,H)
    bias_f = np.zeros((NL, 2, H), np.float32)
    for l, lp in enumerate(gconv):
        bias_f[l, 0] = lp["n1b2"][:H]
        bias_f[l, 1] = lp["n1b2"][H + D:]
    has_free_bias = bool(np.any(bias_f != 0.0))
    has_part_bias = bool(np.any(bias_p != 0.0))
    bias_f = to_bf(bias_f)

    in_maps = []
    for c in range(NCORES):
        m = dict(w)
        m["bias_p"] = bias_p
        m["bias_f"] = bias_f
        m["objembT"] = oe_dev[c]
        m["boxes0T"] = bx0_dev[c]
        m["pvs0T"] = pvs_dev[c]
        m["gs"] = gs_dev[c]
        m["go"] = go_dev[c]
        m["hsc"] = hs_dev[c]
        m["hoc"] = ho_dev[c]
        in_maps.append(m)

    aux = dict(rt=rt, xe=xe, ye=ye, sa=sa, oa=oa, a_m=a_m,
               trip_in=np.asarray(triplets), boxes_gt=boxes_gt,
               has_free_bias=has_free_bias, has_part_bias=has_part_bias)
    return in_maps, aux


# ----------------------------------------------------------------------------
# device graph
# ----------------------------------------------------------------------------

def _build_nc(has_free_bias=False, has_part_bias=False):
    global _NC_CACHE
    key = (has_free_bias, has_part_bias)
    if _NC_CACHE is not None and _NC_CACHE[0] == key:
        return _NC_CACHE[1]

    import concourse.bass as bass
    import concourse.bacc as bacc
    import concourse.tile as tile
    import concourse.mybir as mybir
    from contextlib import ExitStack

    fb = mybir.dt.bfloat16
    f32 = mybir.dt.float32
    RELU = mybir.ActivationFunctionType.Relu
    ADD = mybir.AluOpType.add

    nc = bacc.Bacc("TRN2", target_bir_lowering=False, debug=False,
                   num_devices=NCORES)

    din = {}
    def d_in(name, shape, dt):
        din[name] = nc.dram_tensor(name, list(shape), dt, kind="ExternalInput")

    d_in("objembT", (3, 128, OC), fb)
    d_in("boxes0T", (4, OC), f32)
    d_in("pvs0T", (NT, D, EC), fb)
    d_in("gs", (NT, O, EC), fb)
    d_in("go", (NT, O, EC), fb)
    d_in("hsc", (NT, 112, 4 * O), fb)
    d_in("hoc", (NT, 112, 4 * O), fb)
    d_in("objW1", (3, 128, OBJ_IN), fb)
    d_in("objW1b", (4, OBJ_IN), fb)
    d_in("objW2", (3, 128, OBJ_IN), fb)
    for l in range(NL):
        nk = 7 if l == 0 else 3
        d_in(f"n1w1_{l}", (nk, 128, H), fb)
        d_in(f"n1w2_{l}", (4, 128, 2 * H + D), fb)
        d_in(f"n2w1_{l}", (4, 128, H), fb)
        d_in(f"n2w2_{l}", (4, 128, D), fb)
    d_in("boxw1", (128, H), fb)
    d_in("boxw2", (4, 128, 4), fb)
    d_in("bias_p", (128, 10 * NL + 5), f32)
    d_in("bias_f", (NL, 2, H), fb)

    tovT = nc.dram_tensor("tovT", [NT, 128, OC], fb, kind="ExternalOutput")
    boxpT = nc.dram_tensor("boxpT", [NT, 4, OC], f32, kind="ExternalOutput")

    with tile.TileContext(nc) as tc, ExitStack() as ctx:
        wp = ctx.enter_context(tc.tile_pool(name="wts", bufs=1))
        sp = ctx.enter_context(tc.tile_pool(name="streams", bufs=4))
        ap = ctx.enter_context(tc.tile_pool(name="acts", bufs=2))
        bp = ctx.enter_context(tc.tile_pool(name="boxes", bufs=2))
        pp = ctx.enter_context(tc.tile_pool(name="ps1", bufs=8,
                                            space=bass.MemorySpace.PSUM))

        def wtile(name, shape, dt=fb):
            return wp.tile(list(shape), dt, tag=name, name=name)

        # first-needed constants go on the sync HWDGE queue (ahead of the
        # t=0 streams); the weight bulk goes on the gpsimd SWDGE queue so
        # per-t streams aren't serialized behind ~13MB of weight traffic
        def load(name, nchunks, rows, cols, eng, dt=fb):
            tiles = []
            for k in range(nchunks):
                t = wtile(f"{name}{k}", (rows, cols), dt)
                eng.dma_start(t[:], din[name][k])
                tiles.append(t)
            return tiles

        w_o1 = load("objW1", 3, 128, OBJ_IN, nc.sync)
        w_o1b = wtile("objW1b", (4, OBJ_IN))
        nc.sync.dma_start(w_o1b[:], din["objW1b"][:])
        oembT = wtile("objembT", (128, 3 * OC))
        for k in range(3):
            nc.sync.dma_start(oembT[:, k * OC:(k + 1) * OC], din["objembT"][k])
        w_o2 = load("objW2", 3, 128, OBJ_IN, nc.sync)
        biasp = wtile("bias_p", (128, 10 * NL + 5), f32)
        nc.sync.dma_start(biasp[:], din["bias_p"][:])
        w_n1w1, w_n1w2, w_n2w1, w_n2w2 = [], [], [], []
        for l in range(NL):
            w_n1w1.append(load(f"n1w1_{l}", 7 if l == 0 else 3, 128, H, nc.gpsimd))
            w_n1w2.append(load(f"n1w2_{l}", 4, 128, 2 * H + D, nc.gpsimd))
            w_n2w1.append(load(f"n2w1_{l}", 4, 128, H, nc.gpsimd))
            w_n2w2.append(load(f"n2w2_{l}", 4, 128, D, nc.gpsimd))
        w_bx1 = wtile("boxw1", (128, H))
        nc.gpsimd.dma_start(w_bx1[:], din["boxw1"][:])
        w_bx2 = load("boxw2", 4, 128, 4, nc.gpsimd)
        if has_free_bias:
            biasf = wtile("bias_f", (1, NL * 2 * H), fb)
            nc.gpsimd.dma_start(biasf[:], din["bias_f"][:])
            ones = wtile("ones", (1, 112), fb)
            nc.gpsimd.memset(ones[:], 1.0)

        def msl(m):
            return slice(m * 128, (m + 1) * 128)

        def pbias(col):
            """partition bias AP for activation, or 0.0 when all-zero"""
            return biasp[:, col:col + 1] if has_part_bias else 0.0

        # initial boxes
        boxf = bp.tile([4, OC], f32, tag="boxf")
        nc.sync.dma_start(boxf[:], din["boxes0T"][:])
        boxb = bp.tile([4, OC], fb, tag="boxb")
        nc.vector.tensor_copy(boxb[:], boxf[:])

        for t in range(NT):
            # stream per-t data (sync queue)
            pvsT = sp.tile([D, EC], fb, tag="pvsT")
            nc.sync.dma_start(pvsT[:], din["pvs0T"][t])
            gst = sp.tile([O, EC], fb, tag="gs")
            nc.sync.dma_start(gst[:], din["gs"][t])
            got = sp.tile([O, EC], fb, tag="go")
            nc.sync.dma_start(got[:], din["go"][t])
            hst = sp.tile([112, 4 * O], fb, tag="hs")
            nc.sync.dma_start(hst[:], din["hsc"][t])
            hot = sp.tile([112, 4 * O], fb, tag="ho")
            nc.sync.dma_start(hot[:], din["hoc"][t])

            def open_h1(l, pvsT):
                """open the four h1 psum banks with the full-width pv matmul;
                these have no dependency on the current chain, so they fill
                the PE during boundary stalls"""
                w1p = w_n1w1[l][3 if l == 0 else 1]
                h1ps = [pp.tile([128, EC], f32, tag="ps", name=f"h1ps{m}")
                        for m in range(4)]
                for m in range(4):
                    nc.tensor.matmul(h1ps[m][:], w1p[:, msl(m)], pvsT[:],
                                     start=True, stop=False)
                return h1ps

            h1ps_next = open_h1(0, pvsT)

            # A: ov1T = relu(objW1^T @ [objemb; boxes]); the box matmul is
            # last in each group so obj chunks fill the PE while the previous
            # step's box tail completes
            ov1T = ap.tile([128, 3 * OC], fb, tag="ov1T")
            for m in range(3):
                ps = pp.tile([128, OC], f32, tag="ps")
                for k in range(3):
                    nc.tensor.matmul(ps[:], w_o1[k][:, msl(m)],
                                     oembT[:, k * OC:(k + 1) * OC],
                                     start=(k == 0), stop=False)
                nc.tensor.matmul(ps[:], w_o1b[:, msl(m)], boxb[:],
                                 start=False, stop=True)
                nc.scalar.activation(ov1T[:, m * OC:(m + 1) * OC], ps[:], RELU)

            # B: ov2T = relu(objW2^T @ ov1T)
            ov2T = ap.tile([128, 3 * OC], fb, tag="ov2T")
            for m in range(3):
                ps = pp.tile([128, OC], f32, tag="ps")
                for k in range(3):
                    nc.tensor.matmul(ps[:], w_o2[k][:, msl(m)],
                                     ov1T[:, k * OC:(k + 1) * OC],
                                     start=(k == 0), stop=(k == 2))
                nc.scalar.activation(ov2T[:, m * OC:(m + 1) * OC], ps[:], RELU)


            ovT = ov2T
            for l in range(NL):
                nck = 3 if l == 0 else 1
                w1 = w_n1w1[l]
                w1s, w1o = w1[:nck], w1[nck + 1:]
                h1ps = h1ps_next

                # AsAo = ov @ [W1s | W1o] (natural orientation)
                AsAo = ap.tile([O, 4 * H], fb, tag="AsAo")
                for b in range(BL):
                    for j, wside in enumerate((w1s, w1o)):
                        ps = pp.tile([O, H], f32, tag="ps")
                        for k in range(nck):
                            nc.tensor.matmul(
                                ps[:],
                                ovT[:, k * OC + b * O:k * OC + (b + 1) * O],
                                wside[k][:], start=(k == 0), stop=(k == nck - 1))
                        dsl = slice((2 * b + j) * H, (2 * b + j + 1) * H)
                        if j == 0:
                            nc.vector.tensor_copy(AsAo[:, dsl], ps[:])
                        else:
                            nc.scalar.copy(AsAo[:, dsl], ps[:])

                # gather As/Ao rows onto edges, then bias+relu -> h1T
                h1T = ap.tile([128, 4 * EC], fb, tag="h1T")
                for m in range(4):
                    for b in range(BL):
                        esl = slice(b * EPB, (b + 1) * EPB)
                        nc.tensor.matmul(
                            h1ps[m][:, esl],
                            AsAo[:, 2 * b * H + m * 128:2 * b * H + (m + 1) * 128],
                            gst[:, esl], start=False, stop=False)
                        nc.tensor.matmul(
                            h1ps[m][:, esl],
                            AsAo[:, (2 * b + 1) * H + m * 128:(2 * b + 1) * H + (m + 1) * 128],
                            got[:, esl], start=False, stop=(b == BL - 1))
                    nc.scalar.activation(h1T[:, m * EC:(m + 1) * EC], h1ps[m][:],
                                         RELU, bias=pbias(l * 10 + m))

                # n1W2: new_s/new_o natural per edge-chunk g
                nsno = ap.tile([112, 8 * H], fb, tag="nsno")
                for g in range(4):
                    b, ec = divmod(g, 2)
                    hsl0 = b * EPB + ec * 112
                    for j, c0 in ((0, 0), (1, H + D)):
                        ps = pp.tile([112, H], f32, tag="ps")
                        for k in range(4):
                            nc.tensor.matmul(
                                ps[:],
                                h1T[:, k * EC + hsl0:k * EC + hsl0 + 112],
                                w_n1w2[l][k][:, c0:c0 + H],
                                start=(k == 0), stop=(k == 3 and not has_free_bias))
                        if has_free_bias:
                            bsl = slice((l * 2 + j) * H, (l * 2 + j + 1) * H)
                            nc.tensor.matmul(ps[:], ones[:], biasf[:, bsl],
                                             start=False, stop=True)
                        gsl = slice((2 * g + j) * H, (2 * g + j + 1) * H)
                        if j == 0:
                            nc.vector.tensor_scalar_max(nsno[:, gsl], ps[:], 0.0)
                        else:
                            nc.scalar.activation(nsno[:, gsl], ps[:], RELU)

                if l + 1 < NL:
                    # new_p feeds the next layer's pv term; dead after layer 4
                    pvsN = ap.tile([D, EC], fb, tag="pvsN")
                    ps = pp.tile([128, EC], f32, tag="ps")
                    for k in range(4):
                        nc.tensor.matmul(ps[:], w_n1w2[l][k][:, H:H + D],
                                         h1T[:, k * EC:(k + 1) * EC],
                                         start=(k == 0), stop=(k == 3))
                    if has_part_bias:
                        nc.scalar.activation(pvsN[:], ps[:], RELU,
                                             bias=pbias(l * 10 + 4))
                    else:
                        nc.vector.tensor_scalar_max(pvsN[:], ps[:], 0.0)

                # scatter: pooledT = Hs^T @ new_s + Ho^T @ new_o
                pooledT = ap.tile([128, 4 * OC], fb, tag="pooledT")
                for m in range(4):
                    ps = pp.tile([128, OC], f32, tag="ps")
                    for b in range(BL):
                        osl = slice(b * O, (b + 1) * O)
                        ops = [(so, ht, ec) for so, ht in ((0, hst), (1, hot))
                               for ec in range(2)]
                        for i, (so, ht, ec) in enumerate(ops):
                            g = b * 2 + ec
                            nc.tensor.matmul(
                                ps[:, osl],
                                nsno[:, (2 * g + so) * H + m * 128:
                                     (2 * g + so) * H + (m + 1) * 128],
                                ht[:, g * O:(g + 1) * O],
                                start=(i == 0), stop=(i == len(ops) - 1))
                    if m % 2 == 0:
                        nc.vector.tensor_copy(pooledT[:, m * OC:(m + 1) * OC], ps[:])
                    else:
                        nc.scalar.copy(pooledT[:, m * OC:(m + 1) * OC], ps[:])

                # n2: two dense layers back to ovT
                n2hT = ap.tile([128, 4 * OC], fb, tag="n2hT")
                for m in range(4):
                    ps = pp.tile([128, OC], f32, tag="ps")
                    for k in range(4):
                        nc.tensor.matmul(ps[:], w_n2w1[l][k][:, msl(m)],
                                         pooledT[:, k * OC:(k + 1) * OC],
                                         start=(k == 0), stop=(k == 3))
                    if has_part_bias:
                        nc.scalar.activation(n2hT[:, m * OC:(m + 1) * OC],
                                             ps[:], RELU,
                                             bias=pbias(l * 10 + 5 + m))
                    else:
                        nc.vector.tensor_scalar_max(
                            n2hT[:, m * OC:(m + 1) * OC], ps[:], 0.0)

                # open next layer's h1 banks before the serial ovN stage
                if l + 1 < NL:
                    h1ps_next = open_h1(l + 1, pvsN)

                ovN = ap.tile([128, OC], fb, tag="ovN")
                ps = pp.tile([128, OC], f32, tag="ps")
                for k in range(4):
                    nc.tensor.matmul(ps[:], w_n2w2[l][k][:],
                                     n2hT[:, k * OC:(k + 1) * OC],
                                     start=(k == 0), stop=(k == 3))
                if has_part_bias:
                    nc.scalar.activation(ovN[:], ps[:], RELU,
                                         bias=pbias(l * 10 + 9))
                else:
                    nc.scalar.activation(ovN[:, :O], ps[:, :O], RELU)
                    nc.vector.tensor_scalar_max(ovN[:, O:], ps[:, O:], 0.0)
                ovT = ovN

            # box head
            box1T = ap.tile([128, 4 * OC], fb, tag="box1T")
            for m in range(4):
                ps = pp.tile([128, OC], f32, tag="ps")
                nc.tensor.matmul(ps[:], w_bx1[:, msl(m)], ovT[:],
                                 start=True, stop=True)
                if has_part_bias:
                    nc.scalar.activation(box1T[:, m * OC:(m + 1) * OC], ps[:],
                                         RELU, bias=pbias(50 + m))
                else:
                    nc.vector.tensor_scalar_max(
                        box1T[:, m * OC:(m + 1) * OC], ps[:], 0.0)
            psd = pp.tile([4, OC], f32, tag="ps")
            for k in range(4):
                nc.tensor.matmul(psd[:], w_bx2[k][:],
                                 box1T[:, k * OC:(k + 1) * OC],
                                 start=(k == 0), stop=(k == 3))
            # boxes_t = (delta + boxb2) + boxes_prev, produced in both dtypes
            boxfN = bp.tile([4, OC], f32, tag="boxf")
            nc.vector.scalar_tensor_tensor(boxfN[:], psd[:], biasp[:4, 54:55],
                                           boxf[:], ADD, ADD)
            boxbN = bp.tile([4, OC], fb, tag="boxb")
            nc.vector.scalar_tensor_tensor(boxbN[:], psd[:], biasp[:4, 54:55],
                                           boxf[:], ADD, ADD)
            boxf, boxb = boxfN, boxbN

            nc.sync.dma_start(tovT[t], ovT[:])
            nc.sync.dma_start(boxpT[t], boxf[:])

    nc.compile()
    _NC_CACHE = (key, nc)
    return nc


# ----------------------------------------------------------------------------
# output assembly
# ----------------------------------------------------------------------------

def _assemble(results, aux):
    tovT = np.stack([np.asarray(results[c]["tovT"]) for c in range(NCORES)])
    boxpT = np.stack([np.asarray(results[c]["boxpT"]) for c in range(NCORES)])

    tov = np.zeros((B, F, O, D), np.float32)
    tmp = tovT.astype(np.float32).reshape(NCORES, NT, D, BL, O)
    tov[:, 1:] = tmp.transpose(0, 3, 1, 4, 2).reshape(B, NT, O, D)

    boxes_pred = np.zeros((B, F, O, 4), np.float32)
    boxes_pred[:, 0] = aux["boxes_gt"][:, 0]
    bx = boxpT.reshape(NCORES, NT, 4, BL, O)
    boxes_pred[:, 1:] = bx.transpose(0, 3, 1, 4, 2).reshape(B, NT, O, 4)

    rt = np.ascontiguousarray(aux["rt"], np.float32)
    locs = np.stack([np.broadcast_to(aux["xe"][:, None], (B, F, A)),
                     np.broadcast_to(aux["ye"][:, None], (B, F, A))],
                    -1).astype(np.float32)
    tt = np.stack([np.broadcast_to(aux["sa"][:, None], (B, F, A)),
                   aux["a_m"],
                   np.broadcast_to(aux["oa"][:, None], (B, F, A))],
                  -1).astype(np.int32)
    return tov, boxes_pred, rt, locs, aux["trip_in"], tt


# ----------------------------------------------------------------------------
# entry point
# ----------------------------------------------------------------------------

def kernel(objs=None, triplets=None, actions=None, boxes_gt=None, params=None):
    global LAST_RESULT
    from concourse.bass_utils import run_bass_kernel_spmd

    in_maps, aux = _prep_inputs(objs, triplets, actions, boxes_gt, params)
    nc = _build_nc(aux["has_free_bias"], aux["has_part_bias"])
    res = run_bass_kernel_spmd(nc, in_maps, core_ids=list(range(NCORES)))
    LAST_RESULT = res
    return _assemble(res.results, aux)


# revision 39
# speedup vs baseline: 1.0116x; 1.0116x over previous
"""Trainium2 Bass kernel for nn_Acts2LayoutModel (gnn_message_passing).

Strategy: pure data-parallel over batch (16 batch / 8 cores = 2 per core,
no collectives). All graph gathers/scatters are expressed as one-hot
matmuls with host-precomputed index matrices; all dense layers run in a
feature-major ("transposed") layout so no on-device transposes are needed.
Matmul operands are bf16 (f32 PSUM accumulation); the box recurrence is f32.
"""

import numpy as np
import ml_dtypes

BF = ml_dtypes.bfloat16

# problem shapes (hardcoded per spec)
B, F, O, T, A = 16, 12, 96, 160, 64
D, H = 128, 512
NL = 5
NCORES = 8
BL = B // NCORES          # 2 batch elems per core
OC = BL * O               # 192 object columns
EPB = T + A               # 224 edges per batch elem
EC = BL * EPB             # 448 edge columns
NT = F - 1                # 11 recurrent steps
OBJ_IN = 3 * D            # 384

_NC_CACHE = None
LAST_RESULT = None        # BassKernelResults of the most recent run


# ----------------------------------------------------------------------------
# host-side input preparation
# ----------------------------------------------------------------------------

def _prep_inputs(objs, triplets, actions, boxes_gt, params):
    objs = np.asarray(objs)
    trip = np.asarray(triplets)
    actions = np.asarray(actions, np.float32)
    boxes_gt = np.asarray(boxes_gt, np.float32)

    attr = [np.asarray(params["attr_emb"][i], np.float32) for i in range(3)]
    pred_emb = np.asarray(params["pred_emb"], np.float32)
    acts_emb = np.asarray(params["acts_emb"], np.float32)
    gconv = [{k: np.asarray(v, np.float32) for k, v in lp.items()}
             for lp in params["gconv"]]
    objW1 = np.asarray(params["obj_W1"], np.float32)
    objW2 = np.asarray(params["obj_W2"], np.float32)
    boxW1 = np.asarray(params["box_W1"], np.float32)
    boxW2 = np.asarray(params["box_W2"], np.float32)
    boxb1 = np.asarray(params["box_b1"], np.float32)
    boxb2 = np.asarray(params["box_b2"], np.float32)

    sa = actions[:, :, 0].astype(np.int64)
    a_f = actions[:, :, 1]
    oa = actions[:, :, 2].astype(np.int64)
    f1, f2 = actions[:, :, 3], actions[:, :, 4]
    xe, ye = actions[:, :, 5], actions[:, :, 6]

    tarr = np.arange(F, dtype=np.float32)
    rt = tarr[None, :, None] / F * (f2 - f1 + 1e-6)[:, None, :] + f1[:, None, :]
    inc = (rt >= 0.0) & (rt <= 1.0)
    a_m = np.where(inc, a_f[:, None, :], 0.0)               # (B,F,A)

    # per-step (t = 1..F-1) edge indices
    s_sp, p_sp, o_sp = trip[:, 1:, :, 0], trip[:, 1:, :, 1], trip[:, 1:, :, 2]
    am_t = a_m[:, 1:].astype(np.int64)                       # (B,NT,A)
    s_all = np.concatenate(
        [s_sp, np.broadcast_to(sa[:, None], (B, NT, A))], 2)  # (B,NT,EPB)
    o_all = np.concatenate(
        [o_sp, np.broadcast_to(oa[:, None], (B, NT, A))], 2)
    ind = np.concatenate([p_sp != 0, am_t != 0], 2).astype(np.float32)

    oh_s = (s_all[..., None] == np.arange(O)).astype(np.float32)  # (B,NT,EPB,O)
    oh_o = (o_all[..., None] == np.arange(O)).astype(np.float32)
    cnt = (oh_s * ind[..., None]).sum(2) + (oh_o * ind[..., None]).sum(2)
    scale = 1.0 / np.maximum(cnt, 1.0)                        # (B,NT,O)
    Hs = oh_s * ind[..., None] * scale[:, :, None, :]
    Ho = oh_o * ind[..., None] * scale[:, :, None, :]

    def to_bf(x):
        return np.ascontiguousarray(x.astype(BF))

    # gather matrices, [o, e] layout: (core, t, O, EC)
    def g_dev(oh):
        g = oh.transpose(0, 1, 3, 2)                          # (B,NT,O,EPB)
        g = g.reshape(NCORES, BL, NT, O, EPB).transpose(0, 2, 3, 1, 4)
        return to_bf(g.reshape(NCORES, NT, O, EC))

    gs_dev, go_dev = g_dev(oh_s), g_dev(oh_o)

    # scatter matrices, rows = edge-chunk of 112, col block g=(b,ec): (core,t,112,4*O)
    def h_dev(hh):
        h = hh.reshape(NCORES, BL, NT, 2, 112, O).transpose(0, 2, 4, 1, 3, 5)
        return to_bf(h.reshape(NCORES, NT, 112, 4 * O))

    hs_dev, ho_dev = h_dev(Hs), h_dev(Ho)

    # layer-0 predicate/action vectors, transposed: (core, t, D, EC)
    pv = pred_emb[p_sp]                                       # (B,NT,T,D)
    av = acts_emb[am_t].copy()                                # (B,NT,A,D)
    av[..., D - 3] = np.broadcast_to(xe[:, None], (B, NT, A))
    av[..., D - 2] = np.broadcast_to(ye[:, None], (B, NT, A))
    av[..., D - 1] = rt[:, 1:]
    pvs = np.concatenate([pv, av], 2)                         # (B,NT,EPB,D)
    pvs = pvs.reshape(NCORES, BL, NT, EPB, D).transpose(0, 2, 4, 1, 3)
    pvs_dev = to_bf(pvs.reshape(NCORES, NT, D, EC))

    # object attribute embeddings, transposed: (core, 3, 128, OC)
    obj_emb = np.concatenate([attr[i][objs[..., i]] for i in range(3)], -1)
    oe = obj_emb.reshape(NCORES, BL, O, OBJ_IN).transpose(0, 3, 1, 2)
    oe_dev = to_bf(oe.reshape(NCORES, OBJ_IN, OC).reshape(NCORES, 3, 128, OC))

    # initial boxes, transposed f32: (core, 4, OC)
    bx0 = boxes_gt[:, 0].reshape(NCORES, BL, O, 4).transpose(0, 3, 1, 2)
    bx0_dev = np.ascontiguousarray(bx0.reshape(NCORES, 4, OC), np.float32)

    # weights (replicated across cores)
    w = {}
    w["objW1"] = to_bf(objW1[:OBJ_IN].reshape(3, 128, OBJ_IN))
    w["objW1b"] = to_bf(objW1[OBJ_IN:])
    w["objW2"] = to_bf(objW2.reshape(3, 128, OBJ_IN))
    for l, lp in enumerate(gconv):
        nk = 7 if l == 0 else 3
        w[f"n1w1_{l}"] = to_bf(lp["n1W1"].reshape(nk, 128, H))
        w[f"n1w2_{l}"] = to_bf(lp["n1W2"].reshape(4, 128, 2 * H + D))
        w[f"n2w1_{l}"] = to_bf(lp["n2W1"].reshape(4, 128, H))
        w[f"n2w2_{l}"] = to_bf(lp["n2W2"].reshape(4, 128, D))
    w["boxw1"] = to_bf(boxW1)
    w["boxw2"] = to_bf(boxW2.reshape(4, 128, 4))

    # partition-wise biases: (128, 55) f32
    bias_p = np.zeros((128, 10 * NL + 5), np.float32)
    for l, lp in enumerate(gconv):
        bias_p[:, l * 10:l * 10 + 4] = lp["n1b1"].reshape(4, 128).T
        bias_p[:, l * 10 + 4] = lp["n1b2"][H:H + D]
        bias_p[:, l * 10 + 5:l * 10 + 9] = lp["n2b1"].reshape(4, 128).T
        bias_p[:, l * 10 + 9] = lp["n2b2"]
    bias_p[:, 50:54] = boxb1.reshape(4, 128).T
    bias_p[:4, 54] = boxb2
    bias_p = np.ascontiguousarray(bias_p)

    # free-dim biases, one row each (added via K=1 ones-row matmuls when any
    # is nonzero; the matmuls are skipped entirely otherwise): (NL,2,H)
    bias_f = np.zeros((NL, 2, H), np.float32)
    for l, lp in enumerate(gconv):
        bias_f[l, 0] = lp["n1b2"][:H]
        bias_f[l, 1] = lp["n1b2"][H + D:]
    has_free_bias = bool(np.any(bias_f != 0.0))
    has_part_bias = bool(np.any(bias_p != 0.0))
    bias_f = to_bf(bias_f)

    in_maps = []
    for c in range(NCORES):
        m = dict(w)
        m["bias_p"] = bias_p
        m["bias_f"] = bias_f
        m["objembT"] = oe_dev[c]
        m["boxes0T"] = bx0_dev[c]
        m["pvs0T"] = pvs_dev[c]
        m["gs"] = gs_dev[c]
        m["go"] = go_dev[c]
        m["hsc"] = hs_dev[c]
        m["hoc"] = ho_dev[c]
        in_maps.append(m)

    aux = dict(rt=rt, xe=xe, ye=ye, sa=sa, oa=oa, a_m=a_m,
               trip_in=np.asarray(triplets), boxes_gt=boxes_gt,
               has_free_bias=has_free_bias, has_part_bias=has_part_bias)
    return in_maps, aux


# ----------------------------------------------------------------------------
# device graph
# ----------------------------------------------------------------------------

def _build_nc(has_free_bias=False, has_part_bias=False):
    global _NC_CACHE
    key = (has_free_bias, has_part_bias)
    if _NC_CACHE is not None and _NC_CACHE[0] == key:
        return _NC_CACHE[1]

    import concourse.bass as bass
    import concourse.bacc as bacc
    import concourse.tile as tile
    import concourse.mybir as mybir
    from contextlib import ExitStack

    fb = mybir.dt.bfloat16
    f32 = mybir.dt.float32
    RELU = mybir.ActivationFunctionType.Relu
    ADD = mybir.AluOpType.add

    nc = bacc.Bacc("TRN2", target_bir_lowering=False, debug=False,
                   num_devices=NCORES)

    din = {}
    def d_in(name, shape, dt):
        din[name] = nc.dram_tensor(name, list(shape), dt, kind="ExternalInput")

    d_in("objembT", (3, 128, OC), fb)
    d_in("boxes0T", (4, OC), f32)
    d_in("pvs0T", (NT, D, EC), fb)
    d_in("gs", (NT, O, EC), fb)
    d_in("go", (NT, O, EC), fb)
    d_in("hsc", (NT, 112, 4 * O), fb)
    d_in("hoc", (NT, 112, 4 * O), fb)
    d_in("objW1", (3, 128, OBJ_IN), fb)
    d_in("objW1b", (4, OBJ_IN), fb)
    d_in("objW2", (3, 128, OBJ_IN), fb)
    for l in range(NL):
        nk = 7 if l == 0 else 3
        d_in(f"n1w1_{l}", (nk, 128, H), fb)
        d_in(f"n1w2_{l}", (4, 128, 2 * H + D), fb)
        d_in(f"n2w1_{l}", (4, 128, H), fb)
        d_in(f"n2w2_{l}", (4, 128, D), fb)
    d_in("boxw1", (128, H), fb)
    d_in("boxw2", (4, 128, 4), fb)
    d_in("bias_p", (128, 10 * NL + 5), f32)
    d_in("bias_f", (NL, 2, H), fb)

    tovT = nc.dram_tensor("tovT", [NT, 128, OC], fb, kind="ExternalOutput")
    boxpT = nc.dram_tensor("boxpT", [NT, 4, OC], f32, kind="ExternalOutput")

    with tile.TileContext(nc) as tc, ExitStack() as ctx:
        wp = ctx.enter_context(tc.tile_pool(name="wts", bufs=1))
        sp = ctx.enter_context(tc.tile_pool(name="streams", bufs=4))
        ap = ctx.enter_context(tc.tile_pool(name="acts", bufs=2))
        bp = ctx.enter_context(tc.tile_pool(name="boxes", bufs=2))
        pp = ctx.enter_context(tc.tile_pool(name="ps1", bufs=8,
                                            space=bass.MemorySpace.PSUM))

        def wtile(name, shape, dt=fb):
            return wp.tile(list(shape), dt, tag=name, name=name)

        # first-needed constants go on the sync HWDGE queue (ahead of the
        # t=0 streams); the weight bulk goes on the gpsimd SWDGE queue so
        # per-t streams aren't serialized behind ~13MB of weight traffic
        def load(name, nchunks, rows, cols, eng, dt=fb):
            tiles = []
            for k in range(nchunks):
                t = wtile(f"{name}{k}", (rows, cols), dt)
                eng.dma_start(t[:], din[name][k])
                tiles.append(t)
            return tiles

        w_o1 = load("objW1", 3, 128, OBJ_IN, nc.sync)
        w_o1b = wtile("objW1b", (4, OBJ_IN))
        nc.sync.dma_start(w_o1b[:], din["objW1b"][:])
        oembT = wtile("objembT", (128, 3 * OC))
        for k in range(3):
            nc.sync.dma_start(oembT[:, k * OC:(k + 1) * OC], din["objembT"][k])
        w_o2 = load("objW2", 3, 128, OBJ_IN, nc.sync)
        biasp = wtile("bias_p", (128, 10 * NL + 5), f32)
        nc.sync.dma_start(biasp[:], din["bias_p"][:])
        w_n1w1, w_n1w2, w_n2w1, w_n2w2 = [], [], [], []
        for l in range(NL):
            w_n1w1.append(load(f"n1w1_{l}", 7 if l == 0 else 3, 128, H, nc.gpsimd))
            w_n1w2.append(load(f"n1w2_{l}", 4, 128, 2 * H + D, nc.gpsimd))
            w_n2w1.append(load(f"n2w1_{l}", 4, 128, H, nc.gpsimd))
            w_n2w2.append(load(f"n2w2_{l}", 4, 128, D, nc.gpsimd))
        w_bx1 = wtile("boxw1", (128, H))
        nc.gpsimd.dma_start(w_bx1[:], din["boxw1"][:])
        w_bx2 = load("boxw2", 4, 128, 4, nc.gpsimd)
        if has_free_bias:
            biasf = wtile("bias_f", (1, NL * 2 * H), fb)
            nc.gpsimd.dma_start(biasf[:], din["bias_f"][:])
            ones = wtile("ones", (1, 112), fb)
            nc.gpsimd.memset(ones[:], 1.0)

        def msl(m):
            return slice(m * 128, (m + 1) * 128)

        def pbias(col):
            """partition bias AP for activation, or 0.0 when all-zero"""
            return biasp[:, col:col + 1] if has_part_bias else 0.0

        # initial boxes
        boxf = bp.tile([4, OC], f32, tag="boxf")
        nc.sync.dma_start(boxf[:], din["boxes0T"][:])
        boxb = bp.tile([4, OC], fb, tag="boxb")
        nc.vector.tensor_copy(boxb[:], boxf[:])

        for t in range(NT):
            # stream per-t data (sync queue)
            pvsT = sp.tile([D, EC], fb, tag="pvsT")
            nc.sync.dma_start(pvsT[:], din["pvs0T"][t])
            gst = sp.tile([O, EC], fb, tag="gs")
            nc.sync.dma_start(gst[:], din["gs"][t])
            got = sp.tile([O, EC], fb, tag="go")
            nc.sync.dma_start(got[:], din["go"][t])
            hst = sp.tile([112, 4 * O], fb, tag="hs")
            nc.sync.dma_start(hst[:], din["hsc"][t])
            hot = sp.tile([112, 4 * O], fb, tag="ho")
            nc.sync.dma_start(hot[:], din["hoc"][t])

            def open_h1(l, pvsT):
                """open the four h1 psum banks with the full-width pv matmul;
                these have no dependency on the current chain, so they fill
                the PE during boundary stalls"""
                w1p = w_n1w1[l][3 if l == 0 else 1]
                h1ps = [pp.tile([128, EC], f32, tag="ps", name=f"h1ps{m}")
                        for m in range(4)]
                for m in range(4):
                    nc.tensor.matmul(h1ps[m][:], w1p[:, msl(m)], pvsT[:],
                                     start=True, stop=False)
                return h1ps

            h1ps_next = open_h1(0, pvsT)

            # A: ov1T = relu(objW1^T @ [objemb; boxes]); the box matmul is
            # last in each group so obj chunks fill the PE while the previous
            # step's box tail completes
            ov1T = ap.tile([128, 3 * OC], fb, tag="ov1T")
            for m in range(3):
                ps = pp.tile([128, OC], f32, tag="ps")
                for k in range(3):
                    nc.tensor.matmul(ps[:], w_o1[k][:, msl(m)],
                                     oembT[:, k * OC:(k + 1) * OC],
                                     start=(k == 0), stop=False)
                nc.tensor.matmul(ps[:], w_o1b[:, msl(m)], boxb[:],
                                 start=False, stop=True)
                nc.scalar.activation(ov1T[:, m * OC:(m + 1) * OC], ps[:], RELU)

            # B: ov2T = relu(objW2^T @ ov1T)
            ov2T = ap.tile([128, 3 * OC], fb, tag="ov2T")
            for m in range(3):
                ps = pp.tile([128, OC], f32, tag="ps")
                for k in range(3):
                    nc.tensor.matmul(ps[:], w_o2[k][:, msl(m)],
                                     ov1T[:, k * OC:(k + 1) * OC],
                                     start=(k == 0), stop=(k == 2))
                nc.scalar.activation(ov2T[:, m * OC:(m + 1) * OC], ps[:], RELU)


            ovT = ov2T
            for l in range(NL):
                nck = 3 if l == 0 else 1
                w1 = w_n1w1[l]
                w1s, w1o = w1[:nck], w1[nck + 1:]
                h1ps = h1ps_next

                # AsAo = ov @ [W1s | W1o] (natural orientation)
                AsAo = ap.tile([O, 4 * H], fb, tag="AsAo")
                for b in range(BL):
                    for j, wside in enumerate((w1s, w1o)):
                        ps = pp.tile([O, H], f32, tag="ps")
                        for k in range(nck):
                            nc.tensor.matmul(
                                ps[:],
                                ovT[:, k * OC + b * O:k * OC + (b + 1) * O],
                                wside[k][:], start=(k == 0), stop=(k == nck - 1))
                        dsl = slice((2 * b + j) * H, (2 * b + j + 1) * H)
                        if j == 0:
                            nc.vector.tensor_copy(AsAo[:, dsl], ps[:])
                        else:
                            nc.scalar.copy(AsAo[:, dsl], ps[:])

                # gather As/Ao rows onto edges, then bias+relu -> h1T
                h1T = ap.tile([128, 4 * EC], fb, tag="h1T")
                for m in range(4):
                    for b in range(BL):
                        esl = slice(b * EPB, (b + 1) * EPB)
                        nc.tensor.matmul(
                            h1ps[m][:, esl],
                            AsAo[:, 2 * b * H + m * 128:2 * b * H + (m + 1) * 128],
                            gst[:, esl], start=False, stop=False)
                        nc.tensor.matmul(
                            h1ps[m][:, esl],
                            AsAo[:, (2 * b + 1) * H + m * 128:(2 * b + 1) * H + (m + 1) * 128],
                            got[:, esl], start=False, stop=(b == BL - 1))
                    if has_part_bias or m % 2:
                        nc.scalar.activation(h1T[:, m * EC:(m + 1) * EC],
                                             h1ps[m][:], RELU,
                                             bias=pbias(l * 10 + m))
                    else:
                        nc.vector.tensor_scalar_max(
                            h1T[:, m * EC:(m + 1) * EC], h1ps[m][:], 0.0)

                # n1W2: new_s/new_o natural per edge-chunk g
                nsno = ap.tile([112, 8 * H], fb, tag="nsno")
                for g in range(4):
                    b, ec = divmod(g, 2)
                    hsl0 = b * EPB + ec * 112
                    for j, c0 in ((0, 0), (1, H + D)):
                        ps = pp.tile([112, H], f32, tag="ps")
                        for k in range(4):
                            nc.tensor.matmul(
                                ps[:],
                                h1T[:, k * EC + hsl0:k * EC + hsl0 + 112],
                                w_n1w2[l][k][:, c0:c0 + H],
                                start=(k == 0), stop=(k == 3 and not has_free_bias))
                        if has_free_bias:
                            bsl = slice((l * 2 + j) * H, (l * 2 + j + 1) * H)
                            nc.tensor.matmul(ps[:], ones[:], biasf[:, bsl],
                                             start=False, stop=True)
                        gsl = slice((2 * g + j) * H, (2 * g + j + 1) * H)
                        if j == 0:
                            nc.vector.tensor_scalar_max(nsno[:, gsl], ps[:], 0.0)
                        else:
                            nc.scalar.activation(nsno[:, gsl], ps[:], RELU)

                if l + 1 < NL:
                    # new_p feeds the next layer's pv term; dead after layer 4
                    pvsN = ap.tile([D, EC], fb, tag="pvsN")
                    ps = pp.tile([128, EC], f32, tag="ps")
                    for k in range(4):
                        nc.tensor.matmul(ps[:], w_n1w2[l][k][:, H:H + D],
                                         h1T[:, k * EC:(k + 1) * EC],
                                         start=(k == 0), stop=(k == 3))
                    nc.scalar.activation(pvsN[:], ps[:], RELU,
                                         bias=pbias(l * 10 + 4))

                # scatter: pooledT = Hs^T @ new_s + Ho^T @ new_o
                pooledT = ap.tile([128, 4 * OC], fb, tag="pooledT")
                for m in range(4):
                    ps = pp.tile([128, OC], f32, tag="ps")
                    for b in range(BL):
                        osl = slice(b * O, (b + 1) * O)
                        ops = [(so, ht, ec) for so, ht in ((0, hst), (1, hot))
                               for ec in range(2)]
                        for i, (so, ht, ec) in enumerate(ops):
                            g = b * 2 + ec
                            nc.tensor.matmul(
                                ps[:, osl],
                                nsno[:, (2 * g + so) * H + m * 128:
                                     (2 * g + so) * H + (m + 1) * 128],
                                ht[:, g * O:(g + 1) * O],
                                start=(i == 0), stop=(i == len(ops) - 1))
                    if m % 2 == 0:
                        nc.vector.tensor_copy(pooledT[:, m * OC:(m + 1) * OC], ps[:])
                    else:
                        nc.scalar.copy(pooledT[:, m * OC:(m + 1) * OC], ps[:])

                # n2: two dense layers back to ovT
                n2hT = ap.tile([128, 4 * OC], fb, tag="n2hT")
                for m in range(4):
                    ps = pp.tile([128, OC], f32, tag="ps")
                    for k in range(4):
                        nc.tensor.matmul(ps[:], w_n2w1[l][k][:, msl(m)],
                                         pooledT[:, k * OC:(k + 1) * OC],
                                         start=(k == 0), stop=(k == 3))
                    if has_part_bias:
                        nc.scalar.activation(n2hT[:, m * OC:(m + 1) * OC],
                                             ps[:], RELU,
                                             bias=pbias(l * 10 + 5 + m))
                    else:
                        nc.vector.tensor_scalar_max(
                            n2hT[:, m * OC:(m + 1) * OC], ps[:], 0.0)

                # open next layer's h1 banks before the serial ovN stage
                if l + 1 < NL:
                    h1ps_next = open_h1(l + 1, pvsN)

                ovN = ap.tile([128, OC], fb, tag="ovN")
                ps = pp.tile([128, OC], f32, tag="ps")
                for k in range(4):
                    nc.tensor.matmul(ps[:], w_n2w2[l][k][:],
                                     n2hT[:, k * OC:(k + 1) * OC],
                                     start=(k == 0), stop=(k == 3))
                if has_part_bias:
                    nc.scalar.activation(ovN[:], ps[:], RELU,
                                         bias=pbias(l * 10 + 9))
                else:
                    nc.scalar.activation(ovN[:, :O], ps[:, :O], RELU)
                    nc.vector.tensor_scalar_max(ovN[:, O:], ps[:, O:], 0.0)
                ovT = ovN

            # box head
            box1T = ap.tile([128, 4 * OC], fb, tag="box1T")
            for m in range(4):
                ps = pp.tile([128, OC], f32, tag="ps")
                nc.tensor.matmul(ps[:], w_bx1[:, msl(m)], ovT[:],
                                 start=True, stop=True)
                if has_part_bias:
                    nc.scalar.activation(box1T[:, m * OC:(m + 1) * OC], ps[:],
                                         RELU, bias=pbias(50 + m))
                else:
                    nc.vector.tensor_scalar_max(
                        box1T[:, m * OC:(m + 1) * OC], ps[:], 0.0)
            psd = pp.tile([4, OC], f32, tag="ps")
            for k in range(4):
                nc.tensor.matmul(psd[:], w_bx2[k][:],
                                 box1T[:, k * OC:(k + 1) * OC],
                                 start=(k == 0), stop=(k == 3))
            # boxes_t = (delta + boxb2) + boxes_prev, produced in both dtypes
            boxfN = bp.tile([4, OC], f32, tag="boxf")
            nc.vector.scalar_tensor_tensor(boxfN[:], psd[:], biasp[:4, 54:55],
                                           boxf[:], ADD, ADD)
            boxbN = bp.tile([4, OC], fb, tag="boxb")
            nc.vector.scalar_tensor_tensor(boxbN[:], psd[:], biasp[:4, 54:55],
                                           boxf[:], ADD, ADD)
            boxf, boxb = boxfN, boxbN

            nc.sync.dma_start(tovT[t], ovT[:])
            nc.sync.dma_start(boxpT[t], boxf[:])

    nc.compile()
    _NC_CACHE = (key, nc)
    return nc


# ----------------------------------------------------------------------------
# output assembly
# ----------------------------------------------------------------------------

def _assemble(results, aux):
    tovT = np.stack([np.asarray(results[c]["tovT"]) for c in range(NCORES)])
    boxpT = np.stack([np.asarray(results[c]["boxpT"]) for c in range(NCORES)])

    tov = np.zeros((B, F, O, D), np.float32)
    tmp = tovT.astype(np.float32).reshape(NCORES, NT, D, BL, O)
    tov[:, 1:] = tmp.transpose(0, 3, 1, 4, 2).reshape(B, NT, O, D)

    boxes_pred = np.zeros((B, F, O, 4), np.float32)
    boxes_pred[:, 0] = aux["boxes_gt"][:, 0]
    bx = boxpT.reshape(NCORES, NT, 4, BL, O)
    boxes_pred[:, 1:] = bx.transpose(0, 3, 1, 4, 2).reshape(B, NT, O, 4)

    rt = np.ascontiguousarray(aux["rt"], np.float32)
    locs = np.stack([np.broadcast_to(aux["xe"][:, None], (B, F, A)),
                     np.broadcast_to(aux["ye"][:, None], (B, F, A))],
                    -1).astype(np.float32)
    tt = np.stack([np.broadcast_to(aux["sa"][:, None], (B, F, A)),
                   aux["a_m"],
                   np.broadcast_to(aux["oa"][:, None], (B, F, A))],
                  -1).astype(np.int32)
    return tov, boxes_pred, rt, locs, aux["trip_in"], tt


# ----------------------------------------------------------------------------
# entry point
# ----------------------------------------------------------------------------

def kernel(objs=None, triplets=None, actions=None, boxes_gt=None, params=None):
    global LAST_RESULT
    from concourse.bass_utils import run_bass_kernel_spmd

    in_maps, aux = _prep_inputs(objs, triplets, actions, boxes_gt, params)
    nc = _build_nc(aux["has_free_bias"], aux["has_part_bias"])
    res = run_bass_kernel_spmd(nc, in_maps, core_ids=list(range(NCORES)))
    LAST_RESULT = res
    return _assemble(res.results, aux)
